# revision 5
# baseline (speedup 1.0000x reference)
"""Mistral sparse-MoE (B=4,S=2048,H=1024,F=4096,E=8,top-2) on 8 trn2 cores.

Expert-parallel sharding: core e holds expert e's gate/up/down weights.
The host computes the (tiny) router + top-2 dispatch and uses it to shard:
each core receives exactly the tokens routed to its expert (gathered,
transposed, zero-padded to a common capacity C), the expert weights in
K-major partition-blocked bf16 layout, and the per-token combine weights.
The device kernel computes the full expert FFN
  y = (silu(x@gW^T) * (x@uW^T)) @ dW^T * w
for its tokens; the host scatter-adds the 8 partial outputs back into the
[T, H] result (pure unshard of the expert-parallel partial sums).

DRAM layouts are chunk-contiguous ([nchunks, 128, k, nch] etc.) so every
DMA is a single 8-16KB-per-partition contiguous run. Token capacity C is
rounded to 32 and split into even chunks so no chunk is compute-light
relative to its 16.8MB weight stream. The down-projection weights load is
split into 4 pieces interleaved behind the first chunk's gate/up weight
stream so the tensor engine starts ~10us into the kernel instead of ~48us.
"""

import numpy as np
import ml_dtypes
from contextlib import ExitStack

B, S, H, F, E, TOPK = 4, 2048, 1024, 4096, 8, 2
T = B * S
P = 128
FQ = 512           # f-columns loaded per gate/up weight DMA
NQ = F // FQ       # 8  weight quarters
KH = H // P        # 8  contraction chunks for gate/up
KF = F // P        # 32 contraction chunks for down
HM = H // P        # 8  output row tiles

_BF16 = ml_dtypes.bfloat16

# last program built by kernel() — exposed for external profiling harnesses
_last_nc = None


def _plan_capacity(max_ne):
    """Even token chunks: nchunks ~ C/512, nch multiple of 32."""
    nchunks = max(1, -(-max_ne // 512))
    per = -(-max_ne // nchunks)
    nch = -(-per // 32) * 32
    return nch, nchunks


def _build_program(nch, nchunks, repeat=1):
    import concourse.tile as tile
    from concourse import bacc, mybir

    bf16 = mybir.dt.bfloat16
    f32 = mybir.dt.float32
    C = nch * nchunks

    nc = bacc.Bacc("TRN2", target_bir_lowering=False, debug=False, num_devices=E)

    xT = nc.dram_tensor("xT", [nchunks, P, KH, nch], bf16, kind="ExternalInput").ap()
    gw = nc.dram_tensor("gw", [NQ, P, KH, FQ], bf16, kind="ExternalInput").ap()
    uw = nc.dram_tensor("uw", [NQ, P, KH, FQ], bf16, kind="ExternalInput").ap()
    dw = nc.dram_tensor("dw", [P, KF, H], bf16, kind="ExternalInput").ap()
    wr = nc.dram_tensor("wr", [P, C], f32, kind="ExternalInput").ap()
    yT = nc.dram_tensor("yT", [nchunks, P, HM, nch], bf16, kind="ExternalOutput").ap()

    with tile.TileContext(nc) as tc, ExitStack() as ctx:
        dwp = ctx.enter_context(tc.tile_pool(name="dwp", bufs=1))
        wp = ctx.enter_context(tc.tile_pool(name="wp", bufs=1))
        xp = ctx.enter_context(tc.tile_pool(name="xp", bufs=2))
        gwp = ctx.enter_context(tc.tile_pool(name="gwp", bufs=2))
        uwp = ctx.enter_context(tc.tile_pool(name="uwp", bufs=2))
        hp = ctx.enter_context(tc.tile_pool(name="hp", bufs=2))
        sgp = ctx.enter_context(tc.tile_pool(name="sgp", bufs=4))
        yp = ctx.enter_context(tc.tile_pool(name="yp", bufs=2))
        pg = ctx.enter_context(tc.tile_pool(name="pg", bufs=3, space="PSUM"))
        pu = ctx.enter_context(tc.tile_pool(name="pu", bufs=3, space="PSUM"))
        py = ctx.enter_context(tc.tile_pool(name="py", bufs=2, space="PSUM"))

        dwt = dwp.tile([P, KF, H], bf16)
        wt = wp.tile([P, C], f32)

        first = True
        for n in list(range(nchunks)) * repeat:
            xt = xp.tile([P, KH, nch], bf16)
            if first:
                # split the startup-critical loads so the first matmuls'
                # subtile deps are satisfied before the full tiles land
                nc.sync.dma_start(out=xt[:, 0:2, :], in_=xT[n][:, 0:2, :])
                nc.sync.dma_start(out=xt[:, 2:KH, :], in_=xT[n][:, 2:KH, :])
            else:
                nc.sync.dma_start(out=xt[:], in_=xT[n])

            hts = [None] * KF
            for q in range(NQ):
                gt = gwp.tile([P, KH, FQ], bf16)
                ut = uwp.tile([P, KH, FQ], bf16)
                if first and q == 0:
                    nc.sync.dma_start(out=gt[:, 0:2, :], in_=gw[q][:, 0:2, :])
                    nc.sync.dma_start(out=gt[:, 2:KH, :], in_=gw[q][:, 2:KH, :])
                    nc.sync.dma_start(out=ut[:, 0:4, :], in_=uw[q][:, 0:4, :])
                    nc.sync.dma_start(out=ut[:, 4:KH, :], in_=uw[q][:, 4:KH, :])
                else:
                    nc.sync.dma_start(out=gt[:], in_=gw[q])
                    nc.sync.dma_start(out=ut[:], in_=uw[q])
                if first:
                    # stagger the big resident loads behind the first
                    # chunk's compute-critical weight stream
                    if q == 0:
                        nc.sync.dma_start(out=wt[:], in_=wr[:, :])
                    if q % 2 == 1:
                        j = q // 2
                        nc.sync.dma_start(
                            out=dwt[:, 8 * j:8 * j + 8, :],
                            in_=dw[:, 8 * j:8 * j + 8, :])
                for fm in range(FQ // P):
                    j = q * (FQ // P) + fm
                    psg = pg.tile([P, nch], f32)
                    psu = pu.tile([P, nch], f32)
                    for k in range(KH):
                        nc.tensor.matmul(
                            psg[:], gt[:, k, fm * P:(fm + 1) * P], xt[:, k, :],
                            start=(k == 0), stop=(k == KH - 1))
                    for k in range(KH):
                        nc.tensor.matmul(
                            psu[:], ut[:, k, fm * P:(fm + 1) * P], xt[:, k, :],
                            start=(k == 0), stop=(k == KH - 1))
                    sg = sgp.tile([P, nch], bf16)
                    nc.scalar.activation(
                        sg[:], psg[:], mybir.ActivationFunctionType.Silu)
                    ht = hp.tile([P, nch], bf16, tag=f"h{j}")
                    nc.vector.tensor_mul(ht[:], sg[:], psu[:])
                    hts[j] = ht
            first = False

            yt = yp.tile([P, HM, nch], bf16)
            for hm in range(HM):
                psy = py.tile([P, nch], f32)
                for k in range(KF):
                    nc.tensor.matmul(
                        psy[:], dwt[:, k, hm * P:(hm + 1) * P], hts[k][:],
                        start=(k == 0), stop=(k == KF - 1))
                nc.vector.tensor_mul(
                    yt[:, hm, :], psy[:], wt[:, n * nch:(n + 1) * nch])
            nc.sync.dma_start(out=yT[n], in_=yt[:])

    nc.finalize()
    return nc


def _route(x, router_w):
    # top-2 routing in f64 (exactly ties-stable vs the fp32 reference for
    # any non-degenerate logits)
    logits = x.astype(np.float64) @ router_w.T.astype(np.float64)
    rows = np.arange(T)
    i1 = np.argmax(logits, axis=1)
    v1 = logits[rows, i1]
    masked = logits.copy()
    masked[rows, i1] = -np.inf
    i2 = np.argmax(masked, axis=1)
    v2 = masked[rows, i2]
    e2 = np.exp(v2 - v1)
    w1 = 1.0 / (1.0 + e2)
    w2 = e2 / (1.0 + e2)
    return i1, i2, w1.astype(np.float32), w2.astype(np.float32)


def _pmajor(a, kdim):
    """[K*128, N] -> [128, K, N] partition-major."""
    k, n = a.shape
    return a.reshape(kdim, P, n).transpose(1, 0, 2)


def _wquarters(w):
    """nn.Linear weight [out=F, in=H] -> [NQ, 128, KH, FQ] bf16, where
    quarter q holds columns q*FQ:(q+1)*FQ of w^T in partition-major form."""
    wq = _pmajor(np.asarray(w).T.astype(_BF16), KH)      # [128, KH, F]
    wq = wq.reshape(P, KH, NQ, FQ).transpose(2, 0, 1, 3)  # [NQ, 128, KH, FQ]
    return np.ascontiguousarray(wq)


def _shard_inputs(x_bf, idxs, wts, gate_w, up_w, down_w, nch, nchunks):
    C = nch * nchunks
    in_maps = []
    for e in range(E):
        idx, w = idxs[e], wts[e]
        n_e = len(idx)
        xTe = np.zeros((H, C), dtype=_BF16)
        xTe[:, :n_e] = x_bf[idx].T
        # [128, KH, C] -> chunk-contiguous [nchunks, 128, KH, nch]
        xTe = _pmajor(xTe, KH).reshape(P, KH, nchunks, nch).transpose(2, 0, 1, 3)
        wre = np.zeros((P, C), dtype=np.float32)
        wre[:, :n_e] = w[None, :]
        in_maps.append({
            "xT": np.ascontiguousarray(xTe),
            "gw": _wquarters(np.asarray(gate_w)[e]),
            "uw": _wquarters(np.asarray(up_w)[e]),
            "dw": np.ascontiguousarray(
                _pmajor(np.asarray(down_w)[e].T.astype(_BF16), KF)),
            "wr": wre,
        })
    return in_maps


def kernel(hidden_states, router_w, gate_w, up_w, down_w):
    from concourse.bass_utils import run_bass_kernel_spmd

    x = np.asarray(hidden_states, dtype=np.float32).reshape(T, H)
    router_w = np.asarray(router_w, dtype=np.float32)

    i1, i2, w1, w2 = _route(x, router_w)

    idxs, wts = [], []
    for e in range(E):
        m1 = i1 == e
        m2 = i2 == e
        idx = np.nonzero(m1 | m2)[0]
        w = np.where(m1[idx], w1[idx], w2[idx])
        idxs.append(idx)
        wts.append(w)

    max_ne = max(len(i) for i in idxs)
    nch, nchunks = _plan_capacity(max_ne)

    x_bf = x.astype(_BF16)
    in_maps = _shard_inputs(x_bf, idxs, wts, gate_w, up_w, down_w, nch, nchunks)

    nc = _build_program(nch, nchunks)
    global _last_nc
    _last_nc = nc
    results = run_bass_kernel_spmd(nc, in_maps, list(range(E))).results

    out = np.zeros((T, H), dtype=np.float32)
    for e in range(E):
        idx = idxs[e]
        # yT dram is [nchunks, 128, HM, nch] -> [H, C]
        yTe = (results[e]["yT"].transpose(2, 1, 0, 3)
               .reshape(H, nchunks * nch))
        out[idx] += yTe[:, :len(idx)].astype(np.float32).T
    return out.reshape(B, S, H)
